# revision 2
# baseline (speedup 1.0000x reference)
"""Bass/Trainium2 kernel for nn_CRF_RNN (mean-field CRF iteration), v3.

Math (derived from the reference):
  With orig0[t,n] = 0.01 * sum_f inputs[t,n,f], K2[n,c] = sum_k kernels[n,c,k],
  denom[n] = 0.08 + 0.02 * sum_c K2[n,c], the output is x broadcast over the
  feature dim where x <- (orig0 + 0.02 * (x @ K2^T)) / denom, 3 iterations,
  x0 = orig0.

Distribution: kernels row-sharded over 8 cores (512 rows each). Each core
streams its 64MB slice once. The slice is host-retiled so each
[128c x 512r x 8k] tile is contiguous and column-major: the DVE k-reduction
directly produces K2^T tiles (no PE transposes, fully contiguous DMA reads).
Mean-field iterations are PE matmuls contracting over the full node dim;
x is exchanged between iterations with AllGather collectives. The l22 row
sums ride the stream as ones-stationary PE matmuls; the denominator is
converted to row-partition layout with k=1 matmul transposes so each round's
epilogue is two cheap per-partition DVE ops in the transposed layout that
the next exchange needs anyway.
"""

import numpy as np

# Problem constants (hardcoded per harness contract).
T, N, F, D = 32, 4096, 8, 8
NCORES = 8
S = N // NCORES            # 512 rows per core
NI = S // 128              # 4 row sub-blocks
CT = N // 128              # 32 column tiles
A = 0.01
B = 0.01
RNN = 3

_CACHE = {}


def build_program():
    import concourse.tile as tile
    from concourse import bacc, mybir
    from concourse.masks import make_identity
    from concourse.tile_rust import add_dep_helper
    from contextlib import ExitStack

    dt = mybir.dt.float32
    bf = mybir.dt.bfloat16
    X = mybir.AxisListType.X
    ADD = mybir.AluOpType.add
    COPY = mybir.ActivationFunctionType.Copy

    nc = bacc.Bacc("TRN2", target_bir_lowering=False, debug=False,
                   num_devices=NCORES)
    # host-retiled pairs: kern[cp] = [128 c, 2 tiles, S rows, D k] so each
    # partition line is one contiguous 32KB run
    kern = nc.dram_tensor("kern", [CT // 2, 128, 2, S, D], dt,
                          kind="ExternalInput")
    inp = nc.dram_tensor("inp", [T, S, F], dt, kind="ExternalInput")
    out = nc.dram_tensor("out", [128, NI, T], dt, kind="ExternalOutput")

    with ExitStack() as ctx:
        tc = ctx.enter_context(tile.TileContext(nc))
        singles = ctx.enter_context(tc.tile_pool(name="singles", bufs=1))
        raws = ctx.enter_context(tc.tile_pool(name="raws", bufs=3))
        qraws = ctx.enter_context(tc.tile_pool(name="qraws", bufs=5))
        k2fs = ctx.enter_context(tc.tile_pool(name="k2fs", bufs=3))
        small = ctx.enter_context(tc.tile_pool(name="small", bufs=2))
        tpps = ctx.enter_context(tc.tile_pool(name="tpps", bufs=2, space="PSUM"))
        ypps = ctx.enter_context(tc.tile_pool(name="ypps", bufs=2, space="PSUM"))
        opps = ctx.enter_context(tc.tile_pool(name="opps", bufs=1, space="PSUM"))
        dram = ctx.enter_context(tc.tile_pool(name="dram", bufs=2, space="DRAM"))

        ident = singles.tile([128, 128], dt, tag="ident", name="ident")
        make_identity(nc, ident)
        ones_k = singles.tile([128, 1], bf, tag="ones_k", name="ones_k")
        nc.vector.memset(ones_k, 1.0)
        one_f = singles.tile([1, 1], dt, tag="one_f", name="one_f")
        nc.vector.memset(one_f, 1.0)

        # ---- local feature reduction: ob = A * sum_f inputs ----
        ind = singles.tile([T, S, F], dt, tag="ind", name="ind")
        nc.gpsimd.dma_start(out=ind, in_=inp.ap())
        o_raw = singles.tile([T, S], dt, tag="o_raw", name="o_raw")
        o_raw_red = nc.vector.tensor_reduce(o_raw, ind, axis=X, op=ADD)
        ob = singles.tile([T, S], dt, tag="ob", name="ob")
        nc.scalar.mul(ob, o_raw, float(A))

        # obT[rc, j, t] = ob[t, j*128+rc]
        obT = singles.tile([128, NI, T], dt, tag="obT", name="obT")
        for j in range(NI):
            tp = tpps.tile([128, T], dt, tag="tp", name="tp")
            nc.tensor.transpose(tp, ob[:, j * 128:(j + 1) * 128], ident[:T, :T])
            nc.scalar.copy(obT[:, j, :], tp)
        xs0 = singles.tile([128, NI, T], bf, tag="xs0", name="xs0")
        cp_xs0 = nc.scalar.copy(xs0, obT)

        xr = [singles.tile([128, NCORES, NI, T], bf, tag=f"xr{r}",
                           name=f"xr{r}") for r in range(RNN)]
        xs_tiles = [xs0,
                    singles.tile([128, NI, T], bf, tag="xs1", name="xs1"),
                    singles.tile([128, NI, T], bf, tag="xs2", name="xs2")]

        def exchange(r, eng):
            """AllGather xs_tiles[r] -> xr[r]; returns load instructions."""
            cc_in = dram.tile([128, NI, T], bf, tag=f"cc_in{r}",
                              name=f"cc_in{r}")
            eng.dma_start(out=cc_in, in_=xs_tiles[r])
            cc_out = dram.tile([NCORES, 128, NI, T], bf, tag=f"cc_out{r}",
                               name=f"cc_out{r}")
            nc.gpsimd.collective_compute(
                "AllGather",
                mybir.AluOpType.bypass,
                replica_groups=[list(range(NCORES))],
                ins=[cc_in.opt()],
                outs=[cc_out.opt()],
            )
            xv = xr[r].rearrange("cl (g q) j t -> cl g q j t", g=4)
            cv = cc_out.rearrange("(g q) cl j t -> cl g q j t", g=4)
            lds = [eng.dma_start(out=xv[:, g], in_=cv[:, g]) for g in range(4)]
            return lds

        exchange(0, nc.gpsimd)

        # ---- stream K2T + interleaved l22 / iter-1 matmuls ----
        k2t_all = singles.tile([128, CT, S], bf, tag="k2t", name="k2t_all")
        l22_ps = opps.tile([1, S], dt, tag="l22", name="l22_ps")
        y_ps = ypps.tile([T, S], dt, tag="y", name="y_ps")

        first_k2t_copy = None
        first_red = None
        gate_inst = None
        gate_cp = CT // 4
        y_mms = []
        for cp in range(CT // 2):
            cts = (2 * cp, 2 * cp + 1)
            if cp < CT // 2 - 1:
                # one 4MB DMA covering both tiles of the pair
                raw = raws.tile([128, 2, S, D], dt, tag="rawp", name="raw")
                nc.sync.dma_start(out=raw, in_=kern.ap()[cp])
                k2f = k2fs.tile([128, 2, S], dt, tag="k2fp", name="k2f")
                red = nc.vector.tensor_reduce(k2f, raw, axis=X, op=ADD)
                if first_red is None:
                    first_red = red
                    add_dep_helper(red.ins, o_raw_red.ins, sync=False,
                                   reason="o_raw reduce first on DVE")
                units = [(i, 0, S, k2f[:, i, :]) for i in range(2)]
            else:
                # last pair: quarter granularity to drain the DVE lag fast
                units = []
                for i in range(2):
                    for q in range(4):
                        off = q * 128
                        rawq = qraws.tile([128, 128, D], dt, tag="rawq",
                                          name="rawq")
                        nc.sync.dma_start(
                            out=rawq,
                            in_=kern.ap()[cp, :, i, off:off + 128, :])
                        k2fq = k2fs.tile([128, 128], dt, tag="k2fq",
                                         name="k2fq")
                        nc.vector.tensor_reduce(k2fq, rawq, axis=X, op=ADD)
                        units.append((i, off, 128, k2fq))
            for (i, off, w, src_ap) in units:
                ct = cts[i]
                cpk = nc.scalar.copy(k2t_all[:, ct, off:off + w], src_ap)
                if first_k2t_copy is None:
                    first_k2t_copy = cpk
                    add_dep_helper(cpk.ins, cp_xs0.ins, sync=False,
                                   reason="xs0 copy first on ACT")
                if cp == gate_cp and i == 0 and off == 0:
                    gate_inst = cpk
                nc.tensor.matmul(
                    l22_ps[:, off:off + w], lhsT=ones_k,
                    rhs=k2t_all[:, ct, off:off + w],
                    start=(ct == 0), stop=(ct == CT - 1))
                mm = nc.tensor.matmul(
                    y_ps[:, off:off + w], lhsT=xr[0][:, ct // NI, ct % NI, :],
                    rhs=k2t_all[:, ct, off:off + w],
                    start=(ct == 0), stop=(ct == CT - 1))
                y_mms.append(mm)
        # keep iter-1 matmuls out of the PE stream until AllGather-0 has
        # certainly completed, so the PE FIFO never head-of-line blocks.
        add_dep_helper(y_mms[0].ins, gate_inst.ins, sync=True,
                       reason="defer iter-1 matmuls past mid-stream")

        # ---- denominator (row layout): recips[r] = 1/(0.08 + 0.02*l22[r])
        l22_sb = small.tile([1, S], dt, tag="l22_sb", name="l22_sb")
        nc.scalar.copy(l22_sb, l22_ps)
        l22r = small.tile([128, NI], dt, tag="l22r", name="l22r")
        for q in range(NI):
            trp = tpps.tile([128, T], dt, tag="tp", name="trp")
            nc.tensor.matmul(
                trp[:, 0:1], lhsT=l22_sb[:, q * 128:(q + 1) * 128], rhs=one_f,
                start=True, stop=True)
            nc.scalar.copy(l22r[:, q:q + 1], trp[:, 0:1])
        denom = small.tile([128, NI], dt, tag="denom", name="denom")
        nc.scalar.activation(denom, l22r, COPY,
                             bias=float(A * F), scale=float(2.0 * B))
        recips = small.tile([128, NI], dt, tag="recips", name="recips")
        nc.vector.reciprocal_approx_fast(recips, denom)
        b2r = small.tile([128, NI], dt, tag="b2r", name="b2r")
        nc.scalar.mul(b2r, recips, float(2.0 * B))
        obr = singles.tile([128, NI, T], dt, tag="obr", name="obr")
        for j in range(NI):
            nc.vector.tensor_scalar_mul(
                obr[:, j, :], obT[:, j, :], recips[:, j:j + 1])

        def epilogue(y_psum, dest):
            """dest[:, j, :] = y^T[j] * b2r[:, j] + obr[:, j, :]."""
            y_sb = small.tile([T, S], dt, tag="y_sb", name="y_sb")
            nc.scalar.copy(y_sb, y_psum)
            for j in range(NI):
                ytp = tpps.tile([128, T], dt, tag="tp", name="ytp")
                nc.tensor.transpose(
                    ytp, y_sb[:, j * 128:(j + 1) * 128], ident[:T, :T])
                tmp = small.tile([128, T], dt, tag="ytmp", name="ytmp")
                nc.vector.tensor_scalar_mul(tmp, ytp, b2r[:, j:j + 1])
                nc.vector.tensor_add(dest[:, j, :], tmp, obr[:, j, :])

        # rounds 1..RNN-1: epilogue -> exchange -> next iteration matmuls
        y_cur = y_ps
        for r in range(1, RNN):
            epilogue(y_cur, xs_tiles[r])
            exchange(r, nc.sync)
            y_next = ypps.tile([T, S], dt, tag="y", name="y_ps")
            for ct in range(CT):
                nc.tensor.matmul(
                    y_next, lhsT=xr[r][:, ct // NI, ct % NI, :],
                    rhs=k2t_all[:, ct, :],
                    start=(ct == 0), stop=(ct == CT - 1))
            y_cur = y_next

        out_sb = singles.tile([128, NI, T], dt, tag="out_sb", name="out_sb")
        epilogue(y_cur, out_sb)
        nc.sync.dma_start(out=out.ap(), in_=out_sb)

    nc.compile()
    return nc


# ---------------------------------------------------------------------------
# Host side
# ---------------------------------------------------------------------------

def _get_program():
    if "main" not in _CACHE:
        _CACHE["main"] = build_program()
    return _CACHE["main"]


def make_in_maps(inputs_arr, kernels_arr):
    inputs_arr = np.ascontiguousarray(inputs_arr, dtype=np.float32)
    kernels_arr = np.ascontiguousarray(kernels_arr, dtype=np.float32)
    in_maps = []
    for p in range(NCORES):
        rows = kernels_arr[p * S:(p + 1) * S]          # (S, N, D)
        kt = rows.reshape(S, CT // 2, 2, 128, D).transpose(1, 3, 2, 0, 4)
        kern_p = np.ascontiguousarray(kt, dtype=np.float32)
        inp_p = np.ascontiguousarray(inputs_arr[:, p * S:(p + 1) * S, :])
        in_maps.append({"kern": kern_p, "inp": inp_p})
    return in_maps


def run_device(inputs_arr, kernels_arr, trace=False, tmpdir=None):
    from concourse.bass_utils import run_bass_kernel_spmd

    nc = _get_program()
    in_maps = make_in_maps(inputs_arr, kernels_arr)
    res = run_bass_kernel_spmd(
        nc, in_maps, core_ids=list(range(NCORES)), trace=trace, tmpdir=tmpdir
    )
    # out[p] is x3^T as [128, NI, T]; x3[t, j*128+rc] = out[rc, j, t]
    slices = []
    for p in range(NCORES):
        o = np.asarray(res.results[p]["out"])          # (128, NI, T)
        slices.append(o.transpose(2, 1, 0).reshape(T, S))
    x = np.concatenate(slices, axis=1)                 # (T, N)
    full = np.broadcast_to(x[:, :, None], (T, N, F)).copy()
    return full.astype(np.float32), res


def kernel(**inputs):
    inputs_arr = np.asarray(inputs["inputs"], dtype=np.float32)
    kernels_arr = np.asarray(inputs["kernels"], dtype=np.float32)
    out, _ = run_device(inputs_arr, kernels_arr, trace=False)
    return out
